# revision 20
# baseline (speedup 1.0000x reference)
"""Trainium2 Bass kernel for conformal-prediction interval estimation.

Pipeline (matches the reference nn.Module):
  1. MLP encoder (60 -> 128 -> 128 -> 64) + LayerNorm on test features.
  2. Cosine-similarity attention of encoded queries against the (shared,
     pre-normalized, score-sorted) calibration latents.
  3. Softmax over the calibration axis, cumulative sum, and a weighted
     conformal quantile (searchsorted at 1-alpha) -> per-row interval.
  4. Output (predictions - interval, predictions + interval).

Sharding: data-parallel over the batch. Each of the 8 NeuronCores gets
1024 of the 8192 rows; calibration data and encoder params are replicated.
Host-side glue: batch scatter/gather plus the *shared* calibration
preprocessing (argsort of cal_scores, applying that order to the latents,
unit-normalizing them, and transposing for the matmul layout).

Key kernel trick: because the calibration latents are pre-permuted into
score order, the logits come out of the matmul already sorted; softmax
weights never need an explicit gather. The quantile is then:
    idx = #{k : cumsum(exp)_k < (1-alpha) * total}
computed with a fused scan (initial = -(1-alpha)*total) + is_lt/accum
count, and s_sorted[idx] is fetched with a per-partition indirect DMA.
"""

import os
import sys
from contextlib import ExitStack

sys.path.insert(0, "/opt/trn_rl_repo")
os.environ.setdefault("MYCRO_LOCAL_CACHE", "1")

import numpy as np

import concourse.bass as bass
import concourse.tile as tile
from concourse import bacc, mybir
from concourse.bass_utils import run_bass_kernel_spmd
from concourse.masks import make_identity

N_CORES = 8
BATCH = 8192
ROWS_PER_CORE = BATCH // N_CORES  # 1024
IN_D, HID, LAT = 60, 128, 64
N_CAL = 8192
ALPHA = 0.1
MIN_W, MAX_W = 0.01, 0.2
LN_EPS = 1e-5
P = 128
CHUNK = 512  # matmul free dim == one fp32 PSUM bank
N_CHUNKS = N_CAL // CHUNK  # 16

F32 = mybir.dt.float32
I32 = mybir.dt.int32
ALU = mybir.AluOpType
ACTF = mybir.ActivationFunctionType


def build_program(rows=ROWS_PER_CORE, stage="full"):
    nc = bacc.Bacc(
        "TRN2", target_bir_lowering=False, debug=False, num_devices=N_CORES
    )

    x = nc.dram_tensor("features", [rows, IN_D], F32, kind="ExternalInput").ap()
    pred = nc.dram_tensor("predictions", [rows, 1], F32, kind="ExternalInput").ap()
    cn_t = nc.dram_tensor("cn_t", [LAT, N_CAL], F32, kind="ExternalInput").ap()
    s_srt = nc.dram_tensor("s_sorted", [N_CAL, 1], F32, kind="ExternalInput").ap()
    w1 = nc.dram_tensor("w1", [IN_D, HID], F32, kind="ExternalInput").ap()
    b1 = nc.dram_tensor("b1", [HID, 1], F32, kind="ExternalInput").ap()
    w2 = nc.dram_tensor("w2", [HID, HID], F32, kind="ExternalInput").ap()
    b2 = nc.dram_tensor("b2", [HID, 1], F32, kind="ExternalInput").ap()
    w3 = nc.dram_tensor("w3", [HID, LAT], F32, kind="ExternalInput").ap()
    b3 = nc.dram_tensor("b3", [LAT, 1], F32, kind="ExternalInput").ap()
    ln_w = nc.dram_tensor("ln_w", [1, LAT], F32, kind="ExternalInput").ap()
    ln_b = nc.dram_tensor("ln_b", [1, LAT], F32, kind="ExternalInput").ap()
    lower = nc.dram_tensor("lower", [rows, 1], F32, kind="ExternalOutput").ap()
    upper = nc.dram_tensor("upper", [rows, 1], F32, kind="ExternalOutput").ap()

    n_tiles = rows // P
    ec = min(CHUNK, rows)  # encoder batch-chunk width
    n_ec = rows // ec

    with tile.TileContext(nc) as tc, ExitStack() as ctx:
        const = ctx.enter_context(tc.tile_pool(name="const", bufs=1))
        enc_sb = ctx.enter_context(tc.tile_pool(name="enc_sb", bufs=2))
        ln_sb = ctx.enter_context(tc.tile_pool(name="ln_sb", bufs=2))
        big = ctx.enter_context(tc.tile_pool(name="big", bufs=2))
        small = ctx.enter_context(tc.tile_pool(name="small", bufs=2))
        ps_t = ctx.enter_context(tc.tile_pool(name="ps_t", bufs=2, space="PSUM"))
        ps_mm = ctx.enter_context(tc.tile_pool(name="ps_mm", bufs=2, space="PSUM"))
        ps_at = ctx.enter_context(tc.tile_pool(name="ps_at", bufs=4, space="PSUM"))

        ident = const.tile([P, P], F32)
        make_identity(nc, ident[:])
        zero_b = const.tile([P, 1], F32)
        nc.vector.memset(zero_b[:], 0.0)
        eps_b = const.tile([P, 1], F32)
        nc.vector.memset(eps_b[:], LN_EPS)

        w1s = const.tile([IN_D, HID], F32)
        nc.sync.dma_start(w1s[:], w1[:, :])
        w2s = const.tile([HID, HID], F32)
        nc.sync.dma_start(w2s[:], w2[:, :])
        w3s = const.tile([HID, LAT], F32)
        nc.sync.dma_start(w3s[:], w3[:, :])
        b1s = const.tile([HID, 1], F32)
        nc.sync.dma_start(b1s[:], b1[:, :])
        b2s = const.tile([HID, 1], F32)
        nc.sync.dma_start(b2s[:], b2[:, :])
        b3s = const.tile([LAT, 1], F32)
        nc.sync.dma_start(b3s[:], b3[:, :])
        # ln_w / ln_b broadcast across all partitions (partition-stride 0 read)
        lnw_bc = const.tile([P, LAT], F32)
        nc.sync.dma_start(
            lnw_bc[:],
            bass.AP(tensor=ln_w.tensor, offset=ln_w.offset, ap=[[0, P], [1, LAT]]),
        )
        lnb_bc = const.tile([P, LAT], F32)
        nc.sync.dma_start(
            lnb_bc[:],
            bass.AP(tensor=ln_b.tensor, offset=ln_b.offset, ap=[[0, P], [1, LAT]]),
        )
        cns = const.tile([LAT, N_CAL], F32)
        nc.sync.dma_start(cns[:], cn_t[:, :])
        qnT = const.tile([LAT, rows], F32)

        # ---------------- encoder + layernorm + row-normalize ----------------
        for c in range(n_ec):
            xTs = enc_sb.tile([IN_D, ec], F32, tag="xTs")
            for j in range(ec // P):
                xt = enc_sb.tile([P, IN_D], F32, tag="xt")
                r0 = c * ec + j * P
                nc.sync.dma_start(xt[:], x[r0 : r0 + P, :])
                # each transpose gets a whole PSUM tile: matmul writes at
                # sub-bank free offsets crash the HW path
                xTp = ps_t.tile([IN_D, P], F32, tag="tp")
                nc.tensor.transpose(out=xTp[:], in_=xt[:], identity=ident[:])
                nc.scalar.copy(xTs[:, j * P : (j + 1) * P], xTp[:])
            if stage == "xT":
                nc.sync.dma_start(lower[c * ec : c * ec + IN_D, :], xTs[:, 0:1])
                continue

            h1p = ps_mm.tile([HID, ec], F32, tag="mm")
            nc.tensor.matmul(h1p[:], lhsT=w1s[:], rhs=xTs[:], start=True, stop=True)
            h1 = enc_sb.tile([HID, ec], F32, tag="h1")
            nc.scalar.activation(h1[:], h1p[:], ACTF.Relu, bias=b1s[:])
            if stage == "mm1":
                nc.sync.dma_start(lower[c * ec : c * ec + HID, :], h1[:, 0:1])
                continue

            h2p = ps_mm.tile([HID, ec], F32, tag="mm")
            nc.tensor.matmul(h2p[:], lhsT=w2s[:], rhs=h1[:], start=True, stop=True)
            h2 = enc_sb.tile([HID, ec], F32, tag="h2")
            nc.scalar.activation(h2[:], h2p[:], ACTF.Relu, bias=b2s[:])

            zp = ps_mm.tile([LAT, ec], F32, tag="mm")
            nc.tensor.matmul(zp[:], lhsT=w3s[:], rhs=h2[:], start=True, stop=True)
            zT = enc_sb.tile([LAT, ec], F32, tag="zT")
            nc.scalar.activation(zT[:], zp[:], ACTF.Identity, bias=b3s[:])
            if stage == "mm3":
                nc.sync.dma_start(lower[c * ec : c * ec + LAT, :], zT[:, 0:1])
                continue

            for j in range(ec // P):
                ztp = ps_t.tile([P, LAT], F32, tag="tp")
                nc.tensor.transpose(
                    ztp[:],
                    in_=zT[:, j * P : (j + 1) * P],
                    identity=ident[:LAT, :LAT],
                )
                zz = ln_sb.tile([P, LAT], F32, tag="zz")
                nc.scalar.copy(zz[:], ztp[:])

                stats = ln_sb.tile([P, nc.vector.BN_STATS_DIM], F32, tag="stats")
                nc.vector.bn_stats(out=stats[:], in_=zz[:])
                mv = ln_sb.tile([P, nc.vector.BN_AGGR_DIM], F32, tag="mv")
                nc.vector.bn_aggr(out=mv[:], in_=stats[:])
                rstd = ln_sb.tile([P, 1], F32, tag="rstd")
                nc.scalar.activation(rstd[:], mv[:, 1:2], ACTF.Sqrt, bias=eps_b[:])
                nc.vector.reciprocal(rstd[:], rstd[:])
                q = ln_sb.tile([P, LAT], F32, tag="q")
                nc.vector.tensor_scalar(
                    q[:], zz[:], mv[:, 0:1], rstd[:], op0=ALU.subtract, op1=ALU.mult
                )
                if stage == "lnq":
                    r0 = c * ec + j * P
                    nc.sync.dma_start(lower[r0 : r0 + P, :], q[:, 0:1])
                    continue
                q2 = ln_sb.tile([P, LAT], F32, tag="q2")
                nc.vector.tensor_tensor(q2[:], q[:], lnw_bc[:], op=ALU.mult)
                q3 = ln_sb.tile([P, LAT], F32, tag="q3")
                nc.vector.tensor_tensor(q3[:], q2[:], lnb_bc[:], op=ALU.add)

                sq = ln_sb.tile([P, LAT], F32, tag="sq")
                nc.vector.tensor_tensor(sq[:], q3[:], q3[:], op=ALU.mult)
                ss = ln_sb.tile([P, 1], F32, tag="ss")
                nc.vector.tensor_reduce(
                    out=ss[:], in_=sq[:], axis=mybir.AxisListType.X, op=ALU.add
                )
                nrm = ln_sb.tile([P, 1], F32, tag="nrm")
                nc.scalar.activation(nrm[:], ss[:], ACTF.Sqrt, bias=zero_b[:])
                nc.vector.tensor_scalar(nrm[:], nrm[:], 1e-8, None, op0=ALU.add)
                inv = ln_sb.tile([P, 1], F32, tag="inv")
                nc.vector.reciprocal(inv[:], nrm[:])
                qn = ln_sb.tile([P, LAT], F32, tag="qn")
                nc.vector.tensor_scalar(qn[:], q3[:], inv[:], None, op0=ALU.mult)
                if stage == "qn":
                    r0 = c * ec + j * P
                    nc.sync.dma_start(lower[r0 : r0 + P, :], qn[:, 0:1])
                    continue

                qTp = ps_t.tile([LAT, P], F32, tag="tp")
                nc.tensor.transpose(qTp[:], in_=qn[:], identity=ident[:])
                r0 = c * ec + j * P
                nc.scalar.copy(qnT[:, r0 : r0 + P], qTp[:])

        # ------------- attention + softmax + weighted quantile -------------
        if stage == "enc":
            # debug: dump one qn.T column per tile and stop
            for j in range(n_tiles):
                nc.sync.dma_start(
                    lower[j * P : j * P + LAT, :], qnT[:, j * P : j * P + 1]
                )
                nc.sync.dma_start(
                    upper[j * P : j * P + LAT, :], qnT[:, j * P : j * P + 1]
                )
        attn_stages = ("full", "attn", "count")
        for j in range(n_tiles if stage in attn_stages else 0):
            expt = big.tile([P, N_CAL], F32, tag="exp")
            blk = small.tile([P, N_CHUNKS], F32, tag="blk")
            for n in range(N_CHUNKS):
                lp = ps_at.tile([P, CHUNK], F32, tag="lp")
                nc.tensor.matmul(
                    lp[:],
                    lhsT=qnT[:, j * P : (j + 1) * P],
                    rhs=cns[:, n * CHUNK : (n + 1) * CHUNK],
                    start=True,
                    stop=True,
                )
                nc.scalar.activation(
                    expt[:, n * CHUNK : (n + 1) * CHUNK],
                    lp[:],
                    ACTF.Exp,
                    bias=zero_b[:],
                    accum_out=blk[:, n : n + 1],
                )
            tot = small.tile([P, 1], F32, tag="tot")
            nc.vector.tensor_reduce(
                out=tot[:], in_=blk[:], axis=mybir.AxisListType.X, op=ALU.add
            )
            if stage == "attn":
                # debug: dump per-row softmax denominators and stop
                nc.sync.dma_start(lower[j * P : (j + 1) * P, :], tot[:])
                nc.sync.dma_start(upper[j * P : (j + 1) * P, :], tot[:])
                continue
            tneg = small.tile([P, 1], F32, tag="tneg")
            nc.vector.tensor_scalar(
                tneg[:], tot[:], -(1.0 - ALPHA), None, op0=ALU.mult
            )
            # c'[k] = cumsum(exp)[k] - (1-alpha)*total   (monotone increasing)
            csh = big.tile([P, N_CAL], F32, tag="csh")
            nc.vector.tensor_tensor_scan(
                out=csh[:],
                data0=expt[:],
                data1=expt[:],
                initial=tneg[:],
                op0=ALU.add,
                op1=ALU.bypass,
            )
            # idx = #{k : c'[k] < 0} == searchsorted(cum, (1-alpha)*total)
            cnt = small.tile([P, 1], F32, tag="cnt")
            nc.vector.tensor_scalar(
                csh[:], csh[:], 0.0, None, op0=ALU.is_lt, op1=ALU.add,
                accum_out=cnt[:],
            )
            nc.vector.tensor_scalar(
                cnt[:], cnt[:], float(N_CAL - 1), None, op0=ALU.min
            )
            if stage == "count":
                # debug: dump searchsorted counts and stop
                nc.sync.dma_start(lower[j * P : (j + 1) * P, :], cnt[:])
                nc.sync.dma_start(upper[j * P : (j + 1) * P, :], cnt[:])
                continue
            idx = small.tile([P, 1], I32, tag="idx")
            nc.vector.tensor_copy(out=idx[:], in_=cnt[:])
            sval = small.tile([P, 1], F32, tag="sval")
            nc.gpsimd.indirect_dma_start(
                out=sval[:],
                out_offset=None,
                in_=s_srt[:, :],
                in_offset=bass.IndirectOffsetOnAxis(ap=idx[:, 0:1], axis=0),
            )
            nc.vector.tensor_scalar(
                sval[:], sval[:], MIN_W, MAX_W, op0=ALU.max, op1=ALU.min
            )
            pt = small.tile([P, 1], F32, tag="pt")
            nc.sync.dma_start(pt[:], pred[j * P : (j + 1) * P, :])
            lo = small.tile([P, 1], F32, tag="lo")
            up = small.tile([P, 1], F32, tag="up")
            nc.vector.tensor_tensor(lo[:], pt[:], sval[:], op=ALU.subtract)
            nc.vector.tensor_tensor(up[:], pt[:], sval[:], op=ALU.add)
            nc.sync.dma_start(lower[j * P : (j + 1) * P, :], lo[:])
            nc.sync.dma_start(upper[j * P : (j + 1) * P, :], up[:])

    nc.compile()
    return nc


def host_prep(inputs):
    """Shared calibration-side preprocessing + per-core input maps."""
    f32 = np.float32
    feats = np.ascontiguousarray(np.asarray(inputs["features"], dtype=f32))
    preds = np.asarray(inputs["predictions"], dtype=f32).reshape(-1, 1)
    cal_lat = np.asarray(inputs["cal_latents"], dtype=f32)
    cal_sc = np.asarray(inputs["cal_scores"], dtype=f32)

    order = np.argsort(cal_sc, kind="stable")
    s_sorted = np.ascontiguousarray(cal_sc[order].reshape(N_CAL, 1))
    nrm = np.sqrt((cal_lat * cal_lat).sum(axis=1, keepdims=True)).astype(f32)
    cn = (cal_lat / (nrm + f32(1e-8))).astype(f32)
    cn_t = np.ascontiguousarray(cn[order].T)  # [LAT, N_CAL]

    shared = {
        "cn_t": cn_t,
        "s_sorted": s_sorted,
        "w1": np.ascontiguousarray(np.asarray(inputs["W1"], dtype=f32)),
        "b1": np.asarray(inputs["b1"], dtype=f32).reshape(HID, 1),
        "w2": np.ascontiguousarray(np.asarray(inputs["W2"], dtype=f32)),
        "b2": np.asarray(inputs["b2"], dtype=f32).reshape(HID, 1),
        "w3": np.ascontiguousarray(np.asarray(inputs["W3"], dtype=f32)),
        "b3": np.asarray(inputs["b3"], dtype=f32).reshape(LAT, 1),
        "ln_w": np.asarray(inputs["ln_w"], dtype=f32).reshape(1, LAT),
        "ln_b": np.asarray(inputs["ln_b"], dtype=f32).reshape(1, LAT),
    }
    in_maps = []
    for i in range(N_CORES):
        r0, r1 = i * ROWS_PER_CORE, (i + 1) * ROWS_PER_CORE
        m = dict(shared)
        m["features"] = feats[r0:r1]
        m["predictions"] = np.ascontiguousarray(preds[r0:r1])
        in_maps.append(m)
    return in_maps


_PROGRAM_CACHE = {}


def get_program(rows=ROWS_PER_CORE):
    if rows not in _PROGRAM_CACHE:
        _PROGRAM_CACHE[rows] = build_program(rows)
    return _PROGRAM_CACHE[rows]


def run_on_hw(inputs, trace=False, **kw):
    nc = get_program()
    in_maps = host_prep(inputs)
    res = run_bass_kernel_spmd(nc, in_maps, list(range(N_CORES)), trace=trace, **kw)
    lower = np.concatenate(
        [res.results[i]["lower"].reshape(-1) for i in range(N_CORES)]
    )
    upper = np.concatenate(
        [res.results[i]["upper"].reshape(-1) for i in range(N_CORES)]
    )
    return (lower.astype(np.float32), upper.astype(np.float32)), res


def kernel(**inputs):
    out, _ = run_on_hw(inputs, trace=False)
    return out


# revision 28
# speedup vs baseline: 1.1908x; 1.1908x over previous
"""Trainium2 Bass kernel for conformal-prediction interval estimation.

Pipeline (matches the reference nn.Module):
  1. MLP encoder (60 -> 128 -> 128 -> 64) + LayerNorm on test features.
  2. Cosine-similarity attention of encoded queries against the (shared,
     pre-normalized, score-sorted) calibration latents.
  3. Softmax over the calibration axis, cumulative sum, and a weighted
     conformal quantile (searchsorted at 1-alpha) -> per-row interval.
  4. Output (predictions - interval, predictions + interval).

Sharding: data-parallel over the batch. Each of the 8 NeuronCores gets
1024 of the 8192 rows; calibration data and encoder params are replicated.
Host-side glue: batch scatter/gather plus the *shared* calibration
preprocessing (argsort of cal_scores, applying that order to the latents,
unit-normalizing them, and transposing for the matmul layout).

Key kernel trick: because the calibration latents are pre-permuted into
score order, the logits come out of the matmul already sorted; softmax
weights never need an explicit gather. The quantile is then:
    idx = #{k : cumsum(exp)_k < (1-alpha) * total}
computed with a fused scan (initial = -(1-alpha)*total) + is_lt/accum
count, and s_sorted[idx] is fetched with a per-partition indirect DMA.
"""

import os
import sys
from contextlib import ExitStack

sys.path.insert(0, "/opt/trn_rl_repo")
os.environ.setdefault("MYCRO_LOCAL_CACHE", "1")

import numpy as np

import concourse.bass as bass
import concourse.tile as tile
from concourse import bacc, mybir
from concourse.bass_utils import run_bass_kernel_spmd
from concourse.masks import make_identity

N_CORES = 8
BATCH = 8192
ROWS_PER_CORE = BATCH // N_CORES  # 1024
IN_D, HID, LAT = 60, 128, 64
N_CAL = 8192
ALPHA = 0.1
MIN_W, MAX_W = 0.01, 0.2
LN_EPS = 1e-5
P = 128
CHUNK = 512  # matmul free dim == one fp32 PSUM bank
N_CHUNKS = N_CAL // CHUNK  # 16

F32 = mybir.dt.float32
BF16 = mybir.dt.bfloat16
I32 = mybir.dt.int32
ALU = mybir.AluOpType
ACTF = mybir.ActivationFunctionType


def build_program(rows=ROWS_PER_CORE, stage="full"):
    nc = bacc.Bacc(
        "TRN2", target_bir_lowering=False, debug=False, num_devices=N_CORES
    )

    x = nc.dram_tensor("features", [rows, IN_D], F32, kind="ExternalInput").ap()
    pred = nc.dram_tensor("predictions", [rows, 1], F32, kind="ExternalInput").ap()
    cn_t = nc.dram_tensor("cn_t", [LAT, N_CAL], BF16, kind="ExternalInput").ap()
    id_in = nc.dram_tensor("ident", [P, P], F32, kind="ExternalInput").ap()
    s_srt = nc.dram_tensor("s_sorted", [N_CAL, 1], F32, kind="ExternalInput").ap()
    w1 = nc.dram_tensor("w1", [IN_D, HID], F32, kind="ExternalInput").ap()
    b1 = nc.dram_tensor("b1", [HID, 1], F32, kind="ExternalInput").ap()
    w2 = nc.dram_tensor("w2", [HID, HID], F32, kind="ExternalInput").ap()
    b2 = nc.dram_tensor("b2", [HID, 1], F32, kind="ExternalInput").ap()
    w3 = nc.dram_tensor("w3", [HID, LAT], F32, kind="ExternalInput").ap()
    b3 = nc.dram_tensor("b3", [LAT, 1], F32, kind="ExternalInput").ap()
    ln_w = nc.dram_tensor("ln_w", [1, LAT], F32, kind="ExternalInput").ap()
    ln_b = nc.dram_tensor("ln_b", [1, LAT], F32, kind="ExternalInput").ap()
    lower = nc.dram_tensor("lower", [rows, 1], F32, kind="ExternalOutput").ap()
    upper = nc.dram_tensor("upper", [rows, 1], F32, kind="ExternalOutput").ap()

    n_tiles = rows // P
    ec = min(CHUNK, rows)  # encoder batch-chunk width
    n_ec = rows // ec

    with tile.TileContext(nc) as tc, ExitStack() as ctx:
        const = ctx.enter_context(tc.tile_pool(name="const", bufs=1))
        enc_sb = ctx.enter_context(tc.tile_pool(name="enc_sb", bufs=2))
        ln_sb = ctx.enter_context(tc.tile_pool(name="ln_sb", bufs=2))
        big = ctx.enter_context(tc.tile_pool(name="big", bufs=2))
        small = ctx.enter_context(tc.tile_pool(name="small", bufs=2))
        ps_t = ctx.enter_context(tc.tile_pool(name="ps_t", bufs=2, space="PSUM"))
        ps_mm = ctx.enter_context(tc.tile_pool(name="ps_mm", bufs=2, space="PSUM"))
        ps_at = ctx.enter_context(tc.tile_pool(name="ps_at", bufs=4, space="PSUM"))

        ident = const.tile([P, P], F32)
        nc.sync.dma_start(ident[:], id_in[:, :])
        zero_b = const.tile([P, 1], F32)
        nc.vector.memset(zero_b[:], 0.0)
        eps_b = const.tile([P, 1], F32)
        nc.vector.memset(eps_b[:], LN_EPS)

        w1s = const.tile([IN_D, HID], F32)
        nc.sync.dma_start(w1s[:], w1[:, :])
        w2s = const.tile([HID, HID], F32)
        nc.sync.dma_start(w2s[:], w2[:, :])
        w3s = const.tile([HID, LAT], F32)
        nc.sync.dma_start(w3s[:], w3[:, :])
        b1s = const.tile([HID, 1], F32)
        nc.sync.dma_start(b1s[:], b1[:, :])
        b2s = const.tile([HID, 1], F32)
        nc.sync.dma_start(b2s[:], b2[:, :])
        b3s = const.tile([LAT, 1], F32)
        nc.sync.dma_start(b3s[:], b3[:, :])
        # ln_w / ln_b broadcast across all partitions (partition-stride 0 read)
        lnw_bc = const.tile([P, LAT], F32)
        nc.sync.dma_start(
            lnw_bc[:],
            bass.AP(tensor=ln_w.tensor, offset=ln_w.offset, ap=[[0, P], [1, LAT]]),
        )
        lnb_bc = const.tile([P, LAT], F32)
        nc.sync.dma_start(
            lnb_bc[:],
            bass.AP(tensor=ln_b.tensor, offset=ln_b.offset, ap=[[0, P], [1, LAT]]),
        )
        cns = const.tile([LAT, N_CAL], BF16)
        nc.sync.dma_start(cns[:], cn_t[:, :])
        qnT = const.tile([LAT, rows], BF16)

        # ---------------- encoder + layernorm + row-normalize ----------------
        for c in range(n_ec):
            xTs = enc_sb.tile([IN_D, ec], F32, tag="xTs")
            for j in range(ec // P):
                xt = enc_sb.tile([P, IN_D], F32, tag="xt")
                r0 = c * ec + j * P
                nc.sync.dma_start(xt[:], x[r0 : r0 + P, :])
                # each transpose gets a whole PSUM tile: matmul writes at
                # sub-bank free offsets crash the HW path
                xTp = ps_t.tile([IN_D, P], F32, tag="tp")
                nc.tensor.transpose(out=xTp[:], in_=xt[:], identity=ident[:])
                nc.scalar.copy(xTs[:, j * P : (j + 1) * P], xTp[:])
            if stage == "xT":
                nc.sync.dma_start(lower[c * ec : c * ec + IN_D, :], xTs[:, 0:1])
                continue

            h1p = ps_mm.tile([HID, ec], F32, tag="mm")
            nc.tensor.matmul(h1p[:], lhsT=w1s[:], rhs=xTs[:], start=True, stop=True)
            h1 = enc_sb.tile([HID, ec], F32, tag="h1")
            nc.scalar.activation(h1[:], h1p[:], ACTF.Relu, bias=b1s[:])
            if stage == "mm1":
                nc.sync.dma_start(lower[c * ec : c * ec + HID, :], h1[:, 0:1])
                continue

            h2p = ps_mm.tile([HID, ec], F32, tag="mm")
            nc.tensor.matmul(h2p[:], lhsT=w2s[:], rhs=h1[:], start=True, stop=True)
            h2 = enc_sb.tile([HID, ec], F32, tag="h2")
            nc.scalar.activation(h2[:], h2p[:], ACTF.Relu, bias=b2s[:])

            zp = ps_mm.tile([LAT, ec], F32, tag="mm")
            nc.tensor.matmul(zp[:], lhsT=w3s[:], rhs=h2[:], start=True, stop=True)
            zT = enc_sb.tile([LAT, ec], F32, tag="zT")
            nc.scalar.activation(zT[:], zp[:], ACTF.Identity, bias=b3s[:])
            if stage == "mm3":
                nc.sync.dma_start(lower[c * ec : c * ec + LAT, :], zT[:, 0:1])
                continue

            for j in range(ec // P):
                ztp = ps_t.tile([P, LAT], F32, tag="tp")
                nc.tensor.transpose(
                    ztp[:],
                    in_=zT[:, j * P : (j + 1) * P],
                    identity=ident[:LAT, :LAT],
                )
                zz = ln_sb.tile([P, LAT], F32, tag="zz")
                nc.scalar.copy(zz[:], ztp[:])

                stats = ln_sb.tile([P, nc.vector.BN_STATS_DIM], F32, tag="stats")
                nc.vector.bn_stats(out=stats[:], in_=zz[:])
                mv = ln_sb.tile([P, nc.vector.BN_AGGR_DIM], F32, tag="mv")
                nc.vector.bn_aggr(out=mv[:], in_=stats[:])
                rstd = ln_sb.tile([P, 1], F32, tag="rstd")
                nc.scalar.activation(rstd[:], mv[:, 1:2], ACTF.Sqrt, bias=eps_b[:])
                nc.vector.reciprocal(rstd[:], rstd[:])
                q = ln_sb.tile([P, LAT], F32, tag="q")
                nc.vector.tensor_scalar(
                    q[:], zz[:], mv[:, 0:1], rstd[:], op0=ALU.subtract, op1=ALU.mult
                )
                if stage == "lnq":
                    r0 = c * ec + j * P
                    nc.sync.dma_start(lower[r0 : r0 + P, :], q[:, 0:1])
                    continue
                q2 = ln_sb.tile([P, LAT], F32, tag="q2")
                nc.vector.tensor_tensor(q2[:], q[:], lnw_bc[:], op=ALU.mult)
                q3 = ln_sb.tile([P, LAT], F32, tag="q3")
                nc.vector.tensor_tensor(q3[:], q2[:], lnb_bc[:], op=ALU.add)

                sq = ln_sb.tile([P, LAT], F32, tag="sq")
                nc.vector.tensor_tensor(sq[:], q3[:], q3[:], op=ALU.mult)
                ss = ln_sb.tile([P, 1], F32, tag="ss")
                nc.vector.tensor_reduce(
                    out=ss[:], in_=sq[:], axis=mybir.AxisListType.X, op=ALU.add
                )
                nrm = ln_sb.tile([P, 1], F32, tag="nrm")
                nc.scalar.activation(nrm[:], ss[:], ACTF.Sqrt, bias=zero_b[:])
                nc.vector.tensor_scalar(nrm[:], nrm[:], 1e-8, None, op0=ALU.add)
                inv = ln_sb.tile([P, 1], F32, tag="inv")
                nc.vector.reciprocal(inv[:], nrm[:])
                qn = ln_sb.tile([P, LAT], F32, tag="qn")
                nc.vector.tensor_scalar(qn[:], q3[:], inv[:], None, op0=ALU.mult)
                if stage == "qn":
                    r0 = c * ec + j * P
                    nc.sync.dma_start(lower[r0 : r0 + P, :], qn[:, 0:1])
                    continue

                qTp = ps_t.tile([LAT, P], F32, tag="tp")
                nc.tensor.transpose(qTp[:], in_=qn[:], identity=ident[:])
                r0 = c * ec + j * P
                nc.scalar.copy(qnT[:, r0 : r0 + P], qTp[:])

        # ------------- attention + softmax + weighted quantile -------------
        if stage == "enc":
            # debug: dump one qn.T column per tile and stop
            for j in range(n_tiles):
                nc.sync.dma_start(
                    lower[j * P : j * P + LAT, :], qnT[:, j * P : j * P + 1]
                )
                nc.sync.dma_start(
                    upper[j * P : j * P + LAT, :], qnT[:, j * P : j * P + 1]
                )
        attn_stages = ("full", "attn", "count")
        for j in range(n_tiles if stage in attn_stages else 0):
            expt = big.tile([P, N_CAL], BF16, tag="exp")
            blk = small.tile([P, N_CHUNKS], F32, tag="blk")
            for n in range(N_CHUNKS):
                lp = ps_at.tile([P, CHUNK], F32, tag="lp")
                nc.tensor.matmul(
                    lp[:],
                    lhsT=qnT[:, j * P : (j + 1) * P],
                    rhs=cns[:, n * CHUNK : (n + 1) * CHUNK],
                    start=True,
                    stop=True,
                )
                nc.scalar.activation(
                    expt[:, n * CHUNK : (n + 1) * CHUNK],
                    lp[:],
                    ACTF.Exp,
                    bias=zero_b[:],
                    accum_out=blk[:, n : n + 1],
                )
            tot = small.tile([P, 1], F32, tag="tot")
            nc.vector.tensor_reduce(
                out=tot[:], in_=blk[:], axis=mybir.AxisListType.X, op=ALU.add
            )
            if stage == "attn":
                # debug: dump per-row softmax denominators and stop
                nc.sync.dma_start(lower[j * P : (j + 1) * P, :], tot[:])
                nc.sync.dma_start(upper[j * P : (j + 1) * P, :], tot[:])
                continue
            tneg = small.tile([P, 1], F32, tag="tneg")
            nc.vector.tensor_scalar(
                tneg[:], tot[:], -(1.0 - ALPHA), None, op0=ALU.mult
            )
            # c'[k] = cumsum(exp)[k] - (1-alpha)*total   (monotone increasing;
            # scan state is fp32 internally, bf16 output only rounds the
            # comparison against 0 by <1 index slot)
            csh = big.tile([P, N_CAL], BF16, tag="csh")
            nc.vector.tensor_tensor_scan(
                out=csh[:],
                data0=expt[:],
                data1=expt[:],
                initial=tneg[:],
                op0=ALU.add,
                op1=ALU.bypass,
            )
            # idx = #{k : c'[k] < 0} == searchsorted(cum, (1-alpha)*total)
            # mask written into the dead exp tile (bf16 keeps the fast tier)
            cnt = small.tile([P, 1], F32, tag="cnt")
            nc.vector.tensor_scalar(
                expt[:], csh[:], 0.0, None, op0=ALU.is_lt, op1=ALU.add,
                accum_out=cnt[:],
            )
            nc.vector.tensor_scalar(
                cnt[:], cnt[:], float(N_CAL - 1), None, op0=ALU.min
            )
            if stage == "count":
                # debug: dump searchsorted counts and stop
                nc.sync.dma_start(lower[j * P : (j + 1) * P, :], cnt[:])
                nc.sync.dma_start(upper[j * P : (j + 1) * P, :], cnt[:])
                continue
            idx = small.tile([P, 1], I32, tag="idx")
            nc.vector.tensor_copy(out=idx[:], in_=cnt[:])
            sval = small.tile([P, 1], F32, tag="sval")
            nc.gpsimd.indirect_dma_start(
                out=sval[:],
                out_offset=None,
                in_=s_srt[:, :],
                in_offset=bass.IndirectOffsetOnAxis(ap=idx[:, 0:1], axis=0),
            )
            nc.vector.tensor_scalar(
                sval[:], sval[:], MIN_W, MAX_W, op0=ALU.max, op1=ALU.min
            )
            pt = small.tile([P, 1], F32, tag="pt")
            nc.sync.dma_start(pt[:], pred[j * P : (j + 1) * P, :])
            lo = small.tile([P, 1], F32, tag="lo")
            up = small.tile([P, 1], F32, tag="up")
            nc.vector.tensor_tensor(lo[:], pt[:], sval[:], op=ALU.subtract)
            nc.vector.tensor_tensor(up[:], pt[:], sval[:], op=ALU.add)
            nc.sync.dma_start(lower[j * P : (j + 1) * P, :], lo[:])
            nc.sync.dma_start(upper[j * P : (j + 1) * P, :], up[:])

    nc.compile()
    return nc


def host_prep(inputs):
    """Shared calibration-side preprocessing + per-core input maps."""
    f32 = np.float32
    feats = np.ascontiguousarray(np.asarray(inputs["features"], dtype=f32))
    preds = np.asarray(inputs["predictions"], dtype=f32).reshape(-1, 1)
    cal_lat = np.asarray(inputs["cal_latents"], dtype=f32)
    cal_sc = np.asarray(inputs["cal_scores"], dtype=f32)

    import ml_dtypes

    order = np.argsort(cal_sc, kind="stable")
    s_sorted = np.ascontiguousarray(cal_sc[order].reshape(N_CAL, 1))
    nrm = np.sqrt((cal_lat * cal_lat).sum(axis=1, keepdims=True)).astype(f32)
    cn = (cal_lat / (nrm + f32(1e-8))).astype(f32)
    cn_t = np.ascontiguousarray(cn[order].T).astype(ml_dtypes.bfloat16)

    shared = {
        "cn_t": cn_t,
        "ident": np.eye(P, dtype=f32),
        "s_sorted": s_sorted,
        "w1": np.ascontiguousarray(np.asarray(inputs["W1"], dtype=f32)),
        "b1": np.asarray(inputs["b1"], dtype=f32).reshape(HID, 1),
        "w2": np.ascontiguousarray(np.asarray(inputs["W2"], dtype=f32)),
        "b2": np.asarray(inputs["b2"], dtype=f32).reshape(HID, 1),
        "w3": np.ascontiguousarray(np.asarray(inputs["W3"], dtype=f32)),
        "b3": np.asarray(inputs["b3"], dtype=f32).reshape(LAT, 1),
        "ln_w": np.asarray(inputs["ln_w"], dtype=f32).reshape(1, LAT),
        "ln_b": np.asarray(inputs["ln_b"], dtype=f32).reshape(1, LAT),
    }
    in_maps = []
    for i in range(N_CORES):
        r0, r1 = i * ROWS_PER_CORE, (i + 1) * ROWS_PER_CORE
        m = dict(shared)
        m["features"] = feats[r0:r1]
        m["predictions"] = np.ascontiguousarray(preds[r0:r1])
        in_maps.append(m)
    return in_maps


_PROGRAM_CACHE = {}


def get_program(rows=ROWS_PER_CORE):
    if rows not in _PROGRAM_CACHE:
        _PROGRAM_CACHE[rows] = build_program(rows)
    return _PROGRAM_CACHE[rows]


def run_on_hw(inputs, trace=False, **kw):
    nc = get_program()
    in_maps = host_prep(inputs)
    res = run_bass_kernel_spmd(nc, in_maps, list(range(N_CORES)), trace=trace, **kw)
    lower = np.concatenate(
        [res.results[i]["lower"].reshape(-1) for i in range(N_CORES)]
    )
    upper = np.concatenate(
        [res.results[i]["upper"].reshape(-1) for i in range(N_CORES)]
    )
    return (lower.astype(np.float32), upper.astype(np.float32)), res


def kernel(**inputs):
    out, _ = run_on_hw(inputs, trace=False)
    return out
